# revision 2
# baseline (speedup 1.0000x reference)
"""DiffAttn3d Trainium2 kernel (v7).

8-core sharding: core c -> (batch b = c//4, query slice qs = (c%4)*512).
Each core computes its 512-query slice of the full differential-attention
block (all 16 n-heads) and the final output projection for that slice.

v7 over v6 (v6: ACT exp -> DVE bf16 mask-mult -> bf16 AV; engines were
~70% busy, balanced PE/ACT/DVE ~140-148us each):
- Mask is folded ADDITIVELY into the score PSUM by the PE: each score
  accumulation group opens with identity x maskadd (fp8, {0,-28}) per
  512-col bank, then the two QK matmuls accumulate on top. This removes
  the 88us DVE mask multiply entirely; exp of masked scores -> ~e-28 -> 0
  in fp8.
- exp writes fp8e4 directly (scores |s|<~3 so e^s <= ~20 < 240 max; no
  row-max subtraction needed). AV matmuls run fp8 DoubleRow, contracting
  2 key-chunks per instruction (half the PE time of bf16 AV).
- A slice of the exp work (SCHRAUD_KCS key-chunks x 8 pairs) moves from
  ACT to DVE via a one-instruction Schraudolph exp: fp8e4 bits are an
  affine function of s (11.5416*s + 55.628); DVE tensor_scalar computes
  it in f32 and converts to int8 on writeback (round-to-nearest,
  SATURATING: masked scores give y<=-233 -> -128 = 0x80 = -0.0 in fp8).
  This rebalances ACT ~147us -> ~100us against the now-light DVE.
PSUM: 4 (scores x2) + 2 (av) + 1 (out proj) = 7 of 8 banks.
"""

import math
import numpy as np

B, L, IN_DIM, OUT_DIM = 2, 2048, 128, 128
H, DH = 8, 32
ED = H * DH * 2          # 512
NH = 2 * H               # 16 n-heads
DEPTH = 1
LAMBDA_INIT = 0.8 - 0.6 * math.exp(-0.3 * (DEPTH + 1))
EPS = 1e-8

QSL = 512                # queries per core
NKC = L // 128           # 16 key chunks
NQS = QSL // 128         # 4 query subtiles
NCH = 4                  # head chunks: 4 heads per 128 partitions
AVP = 80                 # av rows: 64 v-dims + 16 ones (xbar tile = 16 rows)

MASK_NEG = -28.0         # additive mask value; exp(-28+3) -> 0 in fp8
# Schraudolph fp8e4 exp: bits = round(8*log2e*s + 8*(7-c)), c=0.0465
SCH_A = 8.0 / math.log(2.0)
SCH_B = 8.0 * (7.0 - 0.0465)
# key chunks whose exp runs on DVE (Schraudolph) instead of ACT
SCHRAUD_KCS = (2, 5, 8, 11, 14)

_CACHE = {}


def _build_program():
    import concourse.bass as bass
    import concourse.tile as tile
    from concourse import bacc, mybir

    f32 = mybir.dt.float32
    bf16 = mybir.dt.bfloat16
    fp8 = mybir.dt.float8e4
    i8 = mybir.dt.int8
    u32 = mybir.dt.uint32
    AF = mybir.ActivationFunctionType
    ALU = mybir.AluOpType
    PM = mybir.MatmulPerfMode

    nc = bacc.Bacc("TRN2", target_bir_lowering=False, debug=False,
                   num_devices=8)

    xsT_d = nc.declare_dram_parameter("xsT", [IN_DIM, L], bf16, isOutput=False)
    xqT_d = nc.declare_dram_parameter("xqT", [IN_DIM, QSL], bf16,
                                      isOutput=False)
    mT_d = nc.declare_dram_parameter("maskT", [L, 2 * QSL], fp8,
                                     isOutput=False)
    id_d = nc.declare_dram_parameter("idm", [128, 128], fp8, isOutput=False)
    # w = [Wq4 | Wk4 | Wv]: Wq/Wk packed 4 heads per 128 cols (offsets
    # 0/32/64/96), Wq pre-scaled by DH^-0.5.
    w_d = nc.declare_dram_parameter("w", [IN_DIM, 3 * ED], bf16,
                                    isOutput=False)
    wo_d = nc.declare_dram_parameter("wo", [64, H, OUT_DIM], bf16,
                                     isOutput=False)
    nlam_d = nc.declare_dram_parameter("nlam", [128, 1], f32, isOutput=False)
    out_d = nc.declare_dram_parameter("out", [QSL, OUT_DIM], f32, isOutput=True)

    with tile.TileContext(nc) as tc:
        with (
            tc.tile_pool(name="const", bufs=1) as const,
            tc.tile_pool(name="psA", bufs=2, space=bass.MemorySpace.PSUM) as psA,
            tc.tile_pool(name="avp", bufs=1, space=bass.MemorySpace.PSUM) as avp,
            tc.tile_pool(name="outp", bufs=1, space=bass.MemorySpace.PSUM) as outp,
            tc.tile_pool(name="epool", bufs=2) as epool,
            tc.tile_pool(name="natp", bufs=2) as natp,
            tc.tile_pool(name="tinyp", bufs=4) as tinyp,
            tc.tile_pool(name="tmpp", bufs=3) as tmpp,
        ):
            # ---- constants / weights (all DMA'd pre-packed) ----
            w_sb = const.tile([128, 3 * ED], bf16)
            nc.sync.dma_start(w_sb[:], w_d[:])
            wo_sb = const.tile([64, H, OUT_DIM], bf16)
            nc.sync.dma_start(wo_sb[:], wo_d[:])
            nlam_sb = const.tile([128, 1], f32)
            nc.sync.dma_start(nlam_sb[:], nlam_d[:])
            idm = const.tile([128, 128], fp8)
            nc.sync.dma_start(idm[:], id_d[:])
            xsT = const.tile([128, L], bf16)
            nc.sync.dma_start(xsT[:], xsT_d[:])
            xqT = const.tile([128, QSL], bf16)
            nc.sync.dma_start(xqT[:], xqT_d[:])
            mkadd = const.tile([128, NKC, 2 * QSL], fp8)
            nc.sync.dma_start(mkadd[:], mT_d.rearrange("(kc p) jq -> p kc jq",
                                                       p=128))
            magic = const.tile([128, NQS], u32)
            nc.vector.memset(magic[:], 0x5F3759DF)

            # ---- projections (bf16 in, fp8/bf16 out, fp32 psum) ----
            # Only chunk 0's qT/kT are emitted up front (the minimum to
            # start the exp stream); everything else is deferred into the
            # early phases' kc slots via setup_items.
            qT = const.tile([128, NCH, QSL], bf16)
            kT = const.tile([128, NCH, L], bf16)
            vp = const.tile([128, NKC, H, AVP], fp8)
            nc.vector.memset(vp[:, :, :, 64:AVP], 1.0)

            def proj_q(c):
                def emit():
                    ps = psA.tile([128, 2, QSL], f32, tag="sps", name="ps")
                    nc.tensor.matmul(ps[:, 0, :],
                                     w_sb[:, c * 128:(c + 1) * 128],
                                     xqT[:], start=True, stop=True)
                    nc.vector.tensor_copy(qT[:, c, :], ps[:, 0, :])
                return emit

            def proj_k(c, h2):
                def emit():
                    ps = psA.tile([128, 2, QSL], f32, tag="sps", name="ps")
                    for s in range(2):
                        nc.tensor.matmul(
                            ps[:, s, :],
                            w_sb[:, ED + c * 128:ED + (c + 1) * 128],
                            xsT[:, (h2 * 2 + s) * 512:(h2 * 2 + s + 1) * 512],
                            start=True, stop=True)
                    nc.vector.tensor_copy(
                        kT[:, c, h2 * 1024:(h2 + 1) * 1024],
                        ps[:].rearrange("p s q -> p (s q)"))
                return emit

            def proj_v(st2):
                def emit():
                    ps = psA.tile([128, 2, QSL], f32, tag="sps", name="ps")
                    for s in range(2):
                        st = st2 * 2 + s
                        nc.tensor.matmul(ps[:, s, :],
                                         xsT[:, st * 128:(st + 1) * 128],
                                         w_sb[:, 2 * ED:3 * ED],
                                         start=True, stop=True)
                    for s in range(2):
                        st = st2 * 2 + s
                        nc.vector.tensor_copy(
                            vp[:, st, :, 0:64],
                            ps[:, s, :].rearrange("p (h d) -> p h d", h=H))
                return emit

            for f in (proj_q(0), proj_k(0, 0), proj_k(0, 1)):
                f()
            # deferred: vp during phase 0 (used by AV from phase 1 on),
            # chunk c's qT/kT during phase 2c-2 (used from phase 2c on)
            setup_items = {
                0: [proj_v(s) for s in range(NKC // 2)],
                1: [proj_q(1), proj_k(1, 0), proj_k(1, 1)],
                2: [proj_q(2), proj_k(2, 0), proj_k(2, 1)],
                3: [proj_q(3), proj_k(3, 0), proj_k(3, 1)],
            }

            out_ps = outp.tile([128, NQS, 128], f32, tag="op", name="out_ps")
            n_outmm = [0]

            def pass1(i, kc, eb):
                c, p = divmod(i, 2)
                sps = psA.tile([128, 2, QSL], f32, tag="sps", name="sps")
                # mask-add opens each bank's accumulation group
                for j in range(2):
                    nc.tensor.matmul(sps[:, j, :], idm[:],
                                     mkadd[:, kc, j * QSL:(j + 1) * QSL],
                                     start=True, stop=False,
                                     skip_group_check=True)
                for j in range(2):
                    r = 64 * p + 32 * j
                    nc.tensor.matmul(
                        sps[:, j, :],
                        kT[r:r + 32, c, kc * 128:(kc + 1) * 128],
                        qT[r:r + 32, c, :],
                        start=False, stop=True, tile_position=(r, 0),
                        skip_group_check=True)
                if kc in SCHRAUD_KCS:
                    nc.vector.tensor_scalar(eb.bitcast(i8)[:, kc, :, :],
                                            sps[:], SCH_A, SCH_B,
                                            ALU.mult, ALU.add)
                else:
                    nc.scalar.activation(eb[:, kc, :, :], sps[:], AF.Exp)

            def av_step(i, kp, eb, av):
                for j in range(2):
                    nc.tensor.matmul(av[j][:], vp[:, 2 * kp:2 * kp + 2, i, :],
                                     eb[:, 2 * kp:2 * kp + 2, j, :],
                                     start=(kp == 0), stop=(kp == NKC // 2 - 1),
                                     perf_mode=PM.DoubleRow)

            def epilogue(i, av):
                nat = []
                for j in range(2):
                    a_sb = tmpp.tile([AVP, QSL], bf16, tag=f"a_sb{j}")
                    nc.vector.tensor_copy(a_sb[:], av[j][:])
                    nt = natp.tile([128, NQS, AVP], bf16, tag=f"nat{j}",
                                   name=f"nat{j}")
                    nc.sync.dma_start_transpose(nt[:], a_sb[:])
                    nat.append(nt)

                r0v = tinyp.tile([128, NQS, 1], f32, tag="r0v")
                nc.vector.reciprocal(r0v[:], nat[0][:, :, 64:65])
                r1v = tinyp.tile([128, NQS, 1], f32, tag="r1v")
                nc.vector.reciprocal(r1v[:], nat[1][:, :, 64:65])
                r1p = tinyp.tile([128, NQS, 1], f32, tag="r1p")
                nc.vector.tensor_scalar(r1p[:], r1v[:], nlam_sb[:], None,
                                        ALU.mult)

                t0 = tmpp.tile([128, NQS, 64], f32, tag="t0")
                nc.vector.tensor_tensor(
                    t0[:], nat[0][:, :, 0:64],
                    r0v[:].broadcast_to([128, NQS, 64]), ALU.mult)
                t1 = tmpp.tile([128, NQS, 64], f32, tag="t1")
                nc.vector.tensor_tensor(
                    t1[:], nat[1][:, :, 0:64],
                    r1p[:].broadcast_to([128, NQS, 64]), ALU.mult)
                at4 = tmpp.tile([128, NQS, 64], bf16, tag="at4")
                nc.vector.tensor_tensor(at4[:], t0[:], t1[:], ALU.add)
                sq4 = tmpp.tile([128, NQS, 64], f32, tag="sq4")
                nc.vector.tensor_tensor(sq4[:], at4[:], at4[:], ALU.mult)
                ss4 = tinyp.tile([128, NQS], f32, tag="ss4")
                nc.vector.tensor_reduce(ss4[:], sq4[:],
                                        mybir.AxisListType.X, ALU.add)

                # rr4 = 1/sqrt(ss4/64): fast inverse sqrt on DVE
                msx = tinyp.tile([128, NQS], f32, tag="msx")
                nc.vector.tensor_scalar(msx[:], ss4[:], 1.0 / 64, None,
                                        ALU.mult)
                sh = tinyp.tile([128, NQS], u32, tag="sh")
                nc.vector.tensor_scalar(sh[:], msx[:].bitcast(u32), 1,
                                        None, ALU.logical_shift_right)
                rr4 = tinyp.tile([128, NQS], f32, tag="rr4")
                nc.vector.tensor_tensor(rr4[:].bitcast(u32), magic[:],
                                        sh[:], ALU.subtract)
                nwu = tinyp.tile([128, NQS], f32, tag="nwu")
                nww = tinyp.tile([128, NQS], f32, tag="nww")
                for _ in range(2):
                    nc.vector.tensor_tensor(nwu[:], rr4[:], rr4[:], ALU.mult)
                    nc.vector.scalar_tensor_tensor(
                        nwu[:], nwu[:], 0.5, msx[:], ALU.mult, ALU.mult)
                    nc.vector.tensor_scalar(nww[:], nwu[:], -1.0, 1.5,
                                            ALU.mult, ALU.add)
                    nc.vector.tensor_tensor(rr4[:], rr4[:], nww[:], ALU.mult)

                # at_s cols 64:128 are junk; the transposed junk rows are
                # never read (projection lhsT slice [0:64])
                at_s = tmpp.tile([128, NQS, 128], bf16, tag="at_s")
                nc.vector.tensor_tensor(
                    at_s[:, :, 0:64], at4[:],
                    rr4[:].unsqueeze(2).broadcast_to([128, NQS, 64]),
                    ALU.mult)
                atT = natp.tile([128, NQS, 128], bf16, tag="atT",
                                name=f"atT{i}")
                nc.sync.dma_start_transpose(atT[:], at_s[:])
                for q in range(NQS):
                    # single accumulation group for the whole bank: start
                    # clears has_written bank-wide
                    nc.tensor.matmul(out_ps[:, q, :], atT[0:64, q, :],
                                     wo_sb[:, i, :],
                                     start=(n_outmm[0] == 0),
                                     stop=(n_outmm[0] == H * NQS - 1))
                    n_outmm[0] += 1

            # ---- attention: software-pipelined over 8 pairs ----
            # phase i: pass1(i) + AV(i-1); the last pair's AV runs inline
            # in phase 7 (after its own mask) so there is no drain phase.
            ebufs, avbufs = {}, {}
            for i in range(H + 1):
                if i < H:
                    ebufs[i] = epool.tile([128, NKC, 2, QSL], fp8,
                                          tag="eall", name=f"eall{i}")
                    avbufs[i] = [avp.tile([AVP, QSL], f32, tag=f"av{j}",
                                          name=f"av{j}_{i}")
                                 for j in range(2)]
                items = setup_items.get(i, [])
                for kc in range(NKC):
                    if i < H:
                        pass1(i, kc, ebufs[i])
                    if items and kc % 2 == 0 and kc // 2 < len(items):
                        items[kc // 2]()
                    if i >= 1 and kc % 2 == 1:
                        av_step(i - 1, kc // 2, ebufs[i - 1], avbufs[i - 1])
                if i >= 1:
                    epilogue(i - 1, avbufs[i - 1])
                    del ebufs[i - 1], avbufs[i - 1]

            out_sb = const.tile([128, NQS, 128], f32)
            nc.vector.tensor_copy(out_sb[:], out_ps[:])
            nc.sync.dma_start(out_d.rearrange("(s p) o -> p s o", p=128),
                              out_sb[:])

    nc.compile()
    return nc


def kernel(**inputs):
    import ml_dtypes
    from concourse.bass_utils import run_bass_kernel_spmd

    bfdt = ml_dtypes.bfloat16
    f8dt = ml_dtypes.float8_e4m3

    x = np.asarray(inputs["x"], np.float32)
    mask = np.asarray(inputs["mask_2d"])
    Wq = np.asarray(inputs["Wq"], np.float32)
    Wkv = np.asarray(inputs["Wkv"], np.float32)
    Wout = np.asarray(inputs["Wout"], np.float32)
    lq1 = np.asarray(inputs["lambda_q1"], np.float32)
    lk1 = np.asarray(inputs["lambda_k1"], np.float32)
    lq2 = np.asarray(inputs["lambda_q2"], np.float32)
    lk2 = np.asarray(inputs["lambda_k2"], np.float32)
    gamma = np.asarray(inputs["gamma"], np.float32)

    lam = float(np.exp(np.sum(lq1 * lk1)) - np.exp(np.sum(lq2 * lk2))
                + LAMBDA_INIT)
    Wq_s = (Wq * DH ** -0.5).astype(np.float32)
    Wk = Wkv[:, :ED]
    Wv = Wkv[:, ED:]

    def pack_heads4(Wm):
        # chunk c (128 cols) holds heads 4c..4c+3 at col offsets 0/32/64/96
        out = np.empty((IN_DIM, NCH * 128), np.float32)
        for n in range(NH):
            c, r = divmod(n, 4)
            out[:, c * 128 + r * 32:c * 128 + r * 32 + 32] = \
                Wm[:, n * DH:(n + 1) * DH]
        return out

    W = np.concatenate([pack_heads4(Wq_s), pack_heads4(Wk), Wv],
                       axis=1).astype(bfdt)
    gs = (gamma * (1.0 - LAMBDA_INIT)).astype(np.float32)
    Wog = (Wout * np.tile(gs, H)[:, None])
    wo = np.ascontiguousarray(
        Wog.reshape(H, 64, OUT_DIM).transpose(1, 0, 2)).astype(bfdt)
    nlam = np.full((128, 1), -lam, np.float32)
    idm = np.eye(128, dtype=np.float32).astype(f8dt)

    xsT = [np.ascontiguousarray(x[b, 0].T).astype(bfdt) for b in range(B)]
    # additive mask, [keys, 2*queries] (duplicated for the j=0/1 banks)
    maskT = []
    for b in range(B):
        madd = np.where(mask[b].T, 0.0, MASK_NEG).astype(np.float32)  # [L, L]q
        maskT.append(madd)

    if "nc" not in _CACHE:
        _CACHE["nc"] = _build_program()
    nc = _CACHE["nc"]

    in_maps = []
    for core in range(8):
        b, qc = divmod(core, 4)
        msl = maskT[b][:, qc * QSL:(qc + 1) * QSL]
        mdup = np.ascontiguousarray(
            np.concatenate([msl, msl], axis=1)).astype(f8dt)
        in_maps.append({
            "xsT": xsT[b],
            "xqT": np.ascontiguousarray(
                xsT[b][:, qc * QSL:(qc + 1) * QSL]),
            "maskT": mdup,
            "idm": idm,
            "w": W,
            "wo": wo,
            "nlam": nlam,
        })

    r = run_bass_kernel_spmd(nc, in_maps, list(range(8)))
    _CACHE["last_results"] = r
    res = r.results

    out = np.empty((B, 1, L, OUT_DIM), np.float32)
    for core in range(8):
        b, qc = divmod(core, 4)
        out[b, 0, qc * QSL:(qc + 1) * QSL, :] = res[core]["out"]
    return out


# revision 6
# speedup vs baseline: 1.0568x; 1.0568x over previous
"""DiffAttn3d Trainium2 kernel (v7).

8-core sharding: core c -> (batch b = c//4, query slice qs = (c%4)*512).
Each core computes its 512-query slice of the full differential-attention
block (all 16 n-heads) and the final output projection for that slice.

v7 over v6 (v6: ACT exp -> DVE bf16 mask-mult -> bf16 AV; engines were
~70% busy, balanced PE/ACT/DVE ~140-148us each):
- Mask is folded ADDITIVELY into the score PSUM by the PE: each score
  accumulation group opens with identity x maskadd (fp8, {0,-28}) per
  512-col bank, then the two QK matmuls accumulate on top. This removes
  the 88us DVE mask multiply entirely; exp of masked scores -> ~e-28 -> 0
  in fp8.
- exp writes fp8e4 directly (scores |s|<~3 so e^s <= ~20 < 240 max; no
  row-max subtraction needed). AV matmuls run fp8 DoubleRow, contracting
  2 key-chunks per instruction (half the PE time of bf16 AV).
- A slice of the exp work (SCHRAUD_KCS key-chunks x 8 pairs) moves from
  ACT to DVE via a one-instruction Schraudolph exp: fp8e4 bits are an
  affine function of s (11.5416*s + 55.628); DVE tensor_scalar computes
  it in f32 and converts to int8 on writeback (round-to-nearest,
  SATURATING: masked scores give y<=-233 -> -128 = 0x80 = -0.0 in fp8).
  This rebalances ACT ~147us -> ~100us against the now-light DVE.
PSUM: 4 (scores x2) + 2 (av) + 1 (out proj) = 7 of 8 banks.
"""

import math
import numpy as np

B, L, IN_DIM, OUT_DIM = 2, 2048, 128, 128
H, DH = 8, 32
ED = H * DH * 2          # 512
NH = 2 * H               # 16 n-heads
DEPTH = 1
LAMBDA_INIT = 0.8 - 0.6 * math.exp(-0.3 * (DEPTH + 1))
EPS = 1e-8

QSL = 512                # queries per core
NKC = L // 128           # 16 key chunks
NQS = QSL // 128         # 4 query subtiles
NCH = 4                  # head chunks: 4 heads per 128 partitions
AVP = 80                 # av rows: 64 v-dims + 16 ones (xbar tile = 16 rows)

MASK_NEG = -28.0         # additive mask value; exp(-28+3) -> 0 in fp8
# Schraudolph fp8e4 exp: bits = round(8*log2e*s + 8*(7-c)), c=0.0465
SCH_A = 8.0 / math.log(2.0)
SCH_B = 8.0 * (7.0 - 0.0465)
# key chunks whose exp runs on DVE (Schraudolph) instead of ACT
SCHRAUD_KCS = (1, 4, 6, 9, 11, 14)

_CACHE = {}


def _build_program():
    import concourse.bass as bass
    import concourse.tile as tile
    from concourse import bacc, mybir

    f32 = mybir.dt.float32
    bf16 = mybir.dt.bfloat16
    fp8 = mybir.dt.float8e4
    i8 = mybir.dt.int8
    u32 = mybir.dt.uint32
    AF = mybir.ActivationFunctionType
    ALU = mybir.AluOpType
    PM = mybir.MatmulPerfMode

    nc = bacc.Bacc("TRN2", target_bir_lowering=False, debug=False,
                   num_devices=8)

    xsT_d = nc.declare_dram_parameter("xsT", [IN_DIM, L], bf16, isOutput=False)
    xqT_d = nc.declare_dram_parameter("xqT", [IN_DIM, QSL], bf16,
                                      isOutput=False)
    mT_d = nc.declare_dram_parameter("maskT", [L, 2 * QSL], fp8,
                                     isOutput=False)
    id_d = nc.declare_dram_parameter("idm", [128, 128], fp8, isOutput=False)
    # w = [Wq4 | Wk4 | Wv]: Wq/Wk packed 4 heads per 128 cols (offsets
    # 0/32/64/96), Wq pre-scaled by DH^-0.5.
    w_d = nc.declare_dram_parameter("w", [IN_DIM, 3 * ED], bf16,
                                    isOutput=False)
    wo_d = nc.declare_dram_parameter("wo", [64, H, OUT_DIM], bf16,
                                     isOutput=False)
    nlam_d = nc.declare_dram_parameter("nlam", [128, 1], f32, isOutput=False)
    out_d = nc.declare_dram_parameter("out", [QSL, OUT_DIM], f32, isOutput=True)

    with tile.TileContext(nc) as tc:
        with (
            tc.tile_pool(name="const", bufs=1) as const,
            tc.tile_pool(name="psA", bufs=2, space=bass.MemorySpace.PSUM) as psA,
            tc.tile_pool(name="avp", bufs=1, space=bass.MemorySpace.PSUM) as avp,
            tc.tile_pool(name="outp", bufs=1, space=bass.MemorySpace.PSUM) as outp,
            tc.tile_pool(name="epool", bufs=2) as epool,
            tc.tile_pool(name="natp", bufs=2) as natp,
            tc.tile_pool(name="tinyp", bufs=4) as tinyp,
            tc.tile_pool(name="tmpp", bufs=3) as tmpp,
        ):
            # ---- constants / weights (all DMA'd pre-packed) ----
            w_sb = const.tile([128, 3 * ED], bf16)
            nc.sync.dma_start(w_sb[:], w_d[:])
            wo_sb = const.tile([64, H, OUT_DIM], bf16)
            nc.sync.dma_start(wo_sb[:], wo_d[:])
            nlam_sb = const.tile([128, 1], f32)
            nc.sync.dma_start(nlam_sb[:], nlam_d[:])
            idm = const.tile([128, 128], fp8)
            nc.sync.dma_start(idm[:], id_d[:])
            xsT = const.tile([128, L], bf16)
            nc.sync.dma_start(xsT[:], xsT_d[:])
            xqT = const.tile([128, QSL], bf16)
            nc.sync.dma_start(xqT[:], xqT_d[:])
            mkadd = const.tile([128, NKC, 2 * QSL], fp8)
            nc.sync.dma_start(mkadd[:], mT_d.rearrange("(kc p) jq -> p kc jq",
                                                       p=128))
            magic = const.tile([128, NQS], u32)
            nc.vector.memset(magic[:], 0x5F3759DF)

            # ---- projections (bf16 in, fp8/bf16 out, fp32 psum) ----
            # Only chunk 0's qT/kT are emitted up front (the minimum to
            # start the exp stream); everything else is deferred into the
            # early phases' kc slots via setup_items.
            qT = const.tile([128, NCH, QSL], bf16)
            kT = const.tile([128, NCH, L], bf16)
            vp = const.tile([128, NKC, H, AVP], fp8)
            nc.vector.memset(vp[:, :, :, 64:AVP], 1.0)

            def proj_q(c):
                def emit():
                    ps = psA.tile([128, 2, QSL], f32, tag="sps", name="ps")
                    nc.tensor.matmul(ps[:, 0, :],
                                     w_sb[:, c * 128:(c + 1) * 128],
                                     xqT[:], start=True, stop=True)
                    nc.vector.tensor_copy(qT[:, c, :], ps[:, 0, :])
                return emit

            def proj_k(c, h2):
                def emit():
                    ps = psA.tile([128, 2, QSL], f32, tag="sps", name="ps")
                    for s in range(2):
                        nc.tensor.matmul(
                            ps[:, s, :],
                            w_sb[:, ED + c * 128:ED + (c + 1) * 128],
                            xsT[:, (h2 * 2 + s) * 512:(h2 * 2 + s + 1) * 512],
                            start=True, stop=True)
                    nc.vector.tensor_copy(
                        kT[:, c, h2 * 1024:(h2 + 1) * 1024],
                        ps[:].rearrange("p s q -> p (s q)"))
                return emit

            def proj_v(st2):
                def emit():
                    ps = psA.tile([128, 2, QSL], f32, tag="sps", name="ps")
                    for s in range(2):
                        st = st2 * 2 + s
                        nc.tensor.matmul(ps[:, s, :],
                                         xsT[:, st * 128:(st + 1) * 128],
                                         w_sb[:, 2 * ED:3 * ED],
                                         start=True, stop=True)
                    for s in range(2):
                        st = st2 * 2 + s
                        nc.vector.tensor_copy(
                            vp[:, st, :, 0:64],
                            ps[:, s, :].rearrange("p (h d) -> p h d", h=H))
                return emit

            for f in (proj_q(0), proj_k(0, 0), proj_k(0, 1)):
                f()
            # deferred: vp during phase 0 (used by AV from phase 1 on),
            # chunk c's qT/kT during phase 2c-2 (used from phase 2c on)
            setup_items = {
                0: [proj_v(s) for s in range(NKC // 2)],
                1: [proj_q(1), proj_k(1, 0), proj_k(1, 1)],
                2: [proj_q(2), proj_k(2, 0), proj_k(2, 1)],
                3: [proj_q(3), proj_k(3, 0), proj_k(3, 1)],
            }

            out_ps = outp.tile([128, NQS, 128], f32, tag="op", name="out_ps")
            n_outmm = [0]

            def pass1(i, kc, eb):
                c, p = divmod(i, 2)
                sps = psA.tile([128, 2, QSL], f32, tag="sps", name="sps")
                # QK first (the two row-tiled matmuls run concurrently and
                # open their banks' accumulation groups); the full-array
                # mask-add matmuls accumulate afterwards.
                for j in range(2):
                    r = 64 * p + 32 * j
                    nc.tensor.matmul(
                        sps[:, j, :],
                        kT[r:r + 32, c, kc * 128:(kc + 1) * 128],
                        qT[r:r + 32, c, :],
                        start=True, stop=False, tile_position=(r, 0),
                        skip_group_check=True)
                for j in range(2):
                    nc.tensor.matmul(sps[:, j, :], idm[:],
                                     mkadd[:, kc, j * QSL:(j + 1) * QSL],
                                     start=False, stop=True,
                                     skip_group_check=True)
                if kc in SCHRAUD_KCS:
                    nc.vector.tensor_scalar(eb.bitcast(i8)[:, kc, :, :],
                                            sps[:], SCH_A, SCH_B,
                                            ALU.mult, ALU.add)
                else:
                    nc.scalar.activation(eb[:, kc, :, :], sps[:], AF.Exp)

            def av_step(i, kp, eb, av):
                for j in range(2):
                    nc.tensor.matmul(av[j][:], vp[:, 2 * kp:2 * kp + 2, i, :],
                                     eb[:, 2 * kp:2 * kp + 2, j, :],
                                     start=(kp == 0), stop=(kp == NKC // 2 - 1),
                                     perf_mode=PM.DoubleRow)

            def epilogue(i, av):
                nat = []
                for j in range(2):
                    a_sb = tmpp.tile([AVP, QSL], bf16, tag=f"a_sb{j}")
                    nc.scalar.copy(a_sb[:], av[j][:])
                    nt = natp.tile([128, NQS, AVP], bf16, tag=f"nat{j}",
                                   name=f"nat{j}")
                    nc.sync.dma_start_transpose(nt[:], a_sb[:])
                    nat.append(nt)

                r0v = tinyp.tile([128, NQS, 1], f32, tag="r0v")
                nc.vector.reciprocal(r0v[:], nat[0][:, :, 64:65])
                r1v = tinyp.tile([128, NQS, 1], f32, tag="r1v")
                nc.vector.reciprocal(r1v[:], nat[1][:, :, 64:65])
                r1p = tinyp.tile([128, NQS, 1], f32, tag="r1p")
                nc.vector.tensor_scalar(r1p[:], r1v[:], nlam_sb[:], None,
                                        ALU.mult)

                t0 = tmpp.tile([128, NQS, 64], f32, tag="t0")
                nc.vector.tensor_tensor(
                    t0[:], nat[0][:, :, 0:64],
                    r0v[:].broadcast_to([128, NQS, 64]), ALU.mult)
                t1 = tmpp.tile([128, NQS, 64], f32, tag="t1")
                nc.vector.tensor_tensor(
                    t1[:], nat[1][:, :, 0:64],
                    r1p[:].broadcast_to([128, NQS, 64]), ALU.mult)
                at4 = tmpp.tile([128, NQS, 64], bf16, tag="at4")
                nc.vector.tensor_tensor(at4[:], t0[:], t1[:], ALU.add)
                sq4 = tmpp.tile([128, NQS, 64], f32, tag="sq4")
                nc.vector.tensor_tensor(sq4[:], at4[:], at4[:], ALU.mult)
                ss4 = tinyp.tile([128, NQS], f32, tag="ss4")
                nc.vector.tensor_reduce(ss4[:], sq4[:],
                                        mybir.AxisListType.X, ALU.add)

                # rr4 = 1/sqrt(ss4/64): fast inverse sqrt on DVE
                msx = tinyp.tile([128, NQS], f32, tag="msx")
                nc.vector.tensor_scalar(msx[:], ss4[:], 1.0 / 64, None,
                                        ALU.mult)
                sh = tinyp.tile([128, NQS], u32, tag="sh")
                nc.vector.tensor_scalar(sh[:], msx[:].bitcast(u32), 1,
                                        None, ALU.logical_shift_right)
                rr4 = tinyp.tile([128, NQS], f32, tag="rr4")
                nc.vector.tensor_tensor(rr4[:].bitcast(u32), magic[:],
                                        sh[:], ALU.subtract)
                nwu = tinyp.tile([128, NQS], f32, tag="nwu")
                nww = tinyp.tile([128, NQS], f32, tag="nww")
                for _ in range(1):
                    nc.vector.tensor_tensor(nwu[:], rr4[:], rr4[:], ALU.mult)
                    nc.vector.scalar_tensor_tensor(
                        nwu[:], nwu[:], 0.5, msx[:], ALU.mult, ALU.mult)
                    nc.vector.tensor_scalar(nww[:], nwu[:], -1.0, 1.5,
                                            ALU.mult, ALU.add)
                    nc.vector.tensor_tensor(rr4[:], rr4[:], nww[:], ALU.mult)

                # at_s cols 64:128 are junk; the transposed junk rows are
                # never read (projection lhsT slice [0:64])
                at_s = tmpp.tile([128, NQS, 128], bf16, tag="at_s")
                nc.vector.tensor_tensor(
                    at_s[:, :, 0:64], at4[:],
                    rr4[:].unsqueeze(2).broadcast_to([128, NQS, 64]),
                    ALU.mult)
                atT = natp.tile([128, NQS, 128], bf16, tag="atT",
                                name=f"atT{i}")
                nc.sync.dma_start_transpose(atT[:], at_s[:])
                for q in range(NQS):
                    # single accumulation group for the whole bank: start
                    # clears has_written bank-wide
                    nc.tensor.matmul(out_ps[:, q, :], atT[0:64, q, :],
                                     wo_sb[:, i, :],
                                     start=(n_outmm[0] == 0),
                                     stop=(n_outmm[0] == H * NQS - 1))
                    n_outmm[0] += 1

            # ---- attention: software-pipelined over 8 pairs ----
            # phase i: pass1(i) + AV(i-1); the last pair's AV runs inline
            # in phase 7 (after its own mask) so there is no drain phase.
            ebufs, avbufs = {}, {}
            for i in range(H + 1):
                if i < H:
                    ebufs[i] = epool.tile([128, NKC, 2, QSL], fp8,
                                          tag="eall", name=f"eall{i}")
                    avbufs[i] = [avp.tile([AVP, QSL], f32, tag=f"av{j}",
                                          name=f"av{j}_{i}")
                                 for j in range(2)]
                items = setup_items.get(i, [])
                for kc in range(NKC):
                    if i < H:
                        pass1(i, kc, ebufs[i])
                    if items and kc % 2 == 0 and kc // 2 < len(items):
                        items[kc // 2]()
                    if i >= 1 and kc % 2 == 1:
                        av_step(i - 1, kc // 2, ebufs[i - 1], avbufs[i - 1])
                if i >= 1:
                    epilogue(i - 1, avbufs[i - 1])
                    del ebufs[i - 1], avbufs[i - 1]

            out_sb = const.tile([128, NQS, 128], f32)
            nc.vector.tensor_copy(out_sb[:], out_ps[:])
            nc.sync.dma_start(out_d.rearrange("(s p) o -> p s o", p=128),
                              out_sb[:])

    nc.compile()
    return nc


def kernel(**inputs):
    import ml_dtypes
    from concourse.bass_utils import run_bass_kernel_spmd

    bfdt = ml_dtypes.bfloat16
    f8dt = ml_dtypes.float8_e4m3

    x = np.asarray(inputs["x"], np.float32)
    mask = np.asarray(inputs["mask_2d"])
    Wq = np.asarray(inputs["Wq"], np.float32)
    Wkv = np.asarray(inputs["Wkv"], np.float32)
    Wout = np.asarray(inputs["Wout"], np.float32)
    lq1 = np.asarray(inputs["lambda_q1"], np.float32)
    lk1 = np.asarray(inputs["lambda_k1"], np.float32)
    lq2 = np.asarray(inputs["lambda_q2"], np.float32)
    lk2 = np.asarray(inputs["lambda_k2"], np.float32)
    gamma = np.asarray(inputs["gamma"], np.float32)

    lam = float(np.exp(np.sum(lq1 * lk1)) - np.exp(np.sum(lq2 * lk2))
                + LAMBDA_INIT)
    Wq_s = (Wq * DH ** -0.5).astype(np.float32)
    Wk = Wkv[:, :ED]
    Wv = Wkv[:, ED:]

    def pack_heads4(Wm):
        # chunk c (128 cols) holds heads 4c..4c+3 at col offsets 0/32/64/96
        out = np.empty((IN_DIM, NCH * 128), np.float32)
        for n in range(NH):
            c, r = divmod(n, 4)
            out[:, c * 128 + r * 32:c * 128 + r * 32 + 32] = \
                Wm[:, n * DH:(n + 1) * DH]
        return out

    W = np.concatenate([pack_heads4(Wq_s), pack_heads4(Wk), Wv],
                       axis=1).astype(bfdt)
    gs = (gamma * (1.0 - LAMBDA_INIT)).astype(np.float32)
    Wog = (Wout * np.tile(gs, H)[:, None])
    wo = np.ascontiguousarray(
        Wog.reshape(H, 64, OUT_DIM).transpose(1, 0, 2)).astype(bfdt)
    nlam = np.full((128, 1), -lam, np.float32)
    idm = np.eye(128, dtype=np.float32).astype(f8dt)

    xsT = [np.ascontiguousarray(x[b, 0].T).astype(bfdt) for b in range(B)]
    # additive mask, [keys, 2*queries] (duplicated for the j=0/1 banks)
    maskT = []
    for b in range(B):
        madd = np.where(mask[b].T, 0.0, MASK_NEG).astype(np.float32)  # [L, L]q
        maskT.append(madd)

    if "nc" not in _CACHE:
        _CACHE["nc"] = _build_program()
    nc = _CACHE["nc"]

    in_maps = []
    for core in range(8):
        b, qc = divmod(core, 4)
        msl = maskT[b][:, qc * QSL:(qc + 1) * QSL]
        mdup = np.ascontiguousarray(
            np.concatenate([msl, msl], axis=1)).astype(f8dt)
        in_maps.append({
            "xsT": xsT[b],
            "xqT": np.ascontiguousarray(
                xsT[b][:, qc * QSL:(qc + 1) * QSL]),
            "maskT": mdup,
            "idm": idm,
            "w": W,
            "wo": wo,
            "nlam": nlam,
        })

    r = run_bass_kernel_spmd(nc, in_maps, list(range(8)))
    _CACHE["last_results"] = r
    res = r.results

    out = np.empty((B, 1, L, OUT_DIM), np.float32)
    for core in range(8):
        b, qc = divmod(core, 4)
        out[b, 0, qc * QSL:(qc + 1) * QSL, :] = res[core]["out"]
    return out


# revision 16
# speedup vs baseline: 1.1160x; 1.0560x over previous
"""DiffAttn3d Trainium2 kernel (v7).

8-core sharding: core c -> (batch b = c//4, query slice qs = (c%4)*512).
Each core computes its 512-query slice of the full differential-attention
block (all 16 n-heads) and the final output projection for that slice.

v7 over v6 (v6: ACT exp -> DVE bf16 mask-mult -> bf16 AV; engines were
~70% busy, balanced PE/ACT/DVE ~140-148us each):
- Mask is folded ADDITIVELY into the score PSUM by the PE: each score
  accumulation group opens with identity x maskadd (fp8, {0,-28}) per
  512-col bank, then the two QK matmuls accumulate on top. This removes
  the 88us DVE mask multiply entirely; exp of masked scores -> ~e-28 -> 0
  in fp8.
- exp writes fp8e4 directly (scores |s|<~3 so e^s <= ~20 < 240 max; no
  row-max subtraction needed). AV matmuls run fp8 DoubleRow, contracting
  2 key-chunks per instruction (half the PE time of bf16 AV).
- A slice of the exp work (SCHRAUD_KCS key-chunks x 8 pairs) moves from
  ACT to DVE via a one-instruction Schraudolph exp: fp8e4 bits are an
  affine function of s (11.5416*s + 55.628); DVE tensor_scalar computes
  it in f32 and converts to int8 on writeback (round-to-nearest,
  SATURATING: masked scores give y<=-233 -> -128 = 0x80 = -0.0 in fp8).
  This rebalances ACT ~147us -> ~100us against the now-light DVE.
PSUM: 4 (scores x2) + 2 (av) + 1 (out proj) = 7 of 8 banks.
"""

import math
import numpy as np

B, L, IN_DIM, OUT_DIM = 2, 2048, 128, 128
H, DH = 8, 32
ED = H * DH * 2          # 512
NH = 2 * H               # 16 n-heads
DEPTH = 1
LAMBDA_INIT = 0.8 - 0.6 * math.exp(-0.3 * (DEPTH + 1))
EPS = 1e-8

QSL = 512                # queries per core
NKC = L // 128           # 16 key chunks
NQS = QSL // 128         # 4 query subtiles
NCH = 4                  # head chunks: 4 heads per 128 partitions
AVP = 80                 # av rows: 64 v-dims + 16 ones (xbar tile = 16 rows)

MASK_NEG = -28.0         # additive mask value; exp(-28+3) ~ 1e-11 in bf16
# Schraudolph bf16 exp: bits = round(128*log2e*s + 128*(127-c)), c=0.040
SCH_A = 128.0 / math.log(2.0)
SCH_B = 128.0 * (127.0 - 0.040)
# key chunks whose mask is folded additively into PSUM by the PE (identity
# matmul); the rest are masked multiplicatively on DVE after the exp
PEMASK_KCS = (0, 3, 5, 8, 11, 13)
# key chunks whose exp runs on DVE (Schraudolph) instead of ACT; must be
# a subset of PEMASK_KCS (Schraudolph needs the mask already in PSUM)
SCHRAUD_KCS = (3, 8, 13)

_CACHE = {}


def _build_program():
    import concourse.bass as bass
    import concourse.tile as tile
    from concourse import bacc, mybir

    f32 = mybir.dt.float32
    bf16 = mybir.dt.bfloat16
    fp8 = mybir.dt.float8e4
    i16 = mybir.dt.int16
    u32 = mybir.dt.uint32
    AF = mybir.ActivationFunctionType
    ALU = mybir.AluOpType

    nc = bacc.Bacc("TRN2", target_bir_lowering=False, debug=False,
                   num_devices=8)

    xsT_d = nc.declare_dram_parameter("xsT", [IN_DIM, L], bf16, isOutput=False)
    xqT_d = nc.declare_dram_parameter("xqT", [IN_DIM, QSL], bf16,
                                      isOutput=False)
    mT_d = nc.declare_dram_parameter("maskT", [L, 2 * QSL], fp8,
                                     isOutput=False)
    mf_d = nc.declare_dram_parameter("maskF", [L, QSL], bf16, isOutput=False)
    id_d = nc.declare_dram_parameter("idm", [128, 128], fp8, isOutput=False)
    # w = [Wq4 | Wk4 | Wv]: Wq/Wk packed 4 heads per 128 cols (offsets
    # 0/32/64/96), Wq pre-scaled by DH^-0.5.
    w_d = nc.declare_dram_parameter("w", [IN_DIM, 3 * ED], bf16,
                                    isOutput=False)
    wo_d = nc.declare_dram_parameter("wo", [64, H, OUT_DIM], bf16,
                                     isOutput=False)
    nlam_d = nc.declare_dram_parameter("nlam", [128, 1], f32, isOutput=False)
    out_d = nc.declare_dram_parameter("out", [QSL, OUT_DIM], f32, isOutput=True)

    with tile.TileContext(nc) as tc:
        with (
            tc.tile_pool(name="const", bufs=1) as const,
            tc.tile_pool(name="psA", bufs=2, space=bass.MemorySpace.PSUM) as psA,
            tc.tile_pool(name="avp", bufs=1, space=bass.MemorySpace.PSUM) as avp,
            tc.tile_pool(name="outp", bufs=1, space=bass.MemorySpace.PSUM) as outp,
            tc.tile_pool(name="epool", bufs=2) as epool,
            tc.tile_pool(name="natp", bufs=2) as natp,
            tc.tile_pool(name="tinyp", bufs=4) as tinyp,
            tc.tile_pool(name="tmpp", bufs=3) as tmpp,
        ):
            # ---- constants / weights (all DMA'd pre-packed) ----
            w_sb = const.tile([128, 3 * ED], bf16)
            nc.sync.dma_start(w_sb[:], w_d[:])
            wo_sb = const.tile([64, H, OUT_DIM], bf16)
            nc.sync.dma_start(wo_sb[:], wo_d[:])
            nlam_sb = const.tile([128, 1], f32)
            nc.sync.dma_start(nlam_sb[:], nlam_d[:])
            idm = const.tile([128, 128], fp8)
            nc.sync.dma_start(idm[:], id_d[:])
            xsT = const.tile([128, L], bf16)
            nc.sync.dma_start(xsT[:], xsT_d[:])
            xqT = const.tile([128, QSL], bf16)
            nc.sync.dma_start(xqT[:], xqT_d[:])
            mkadd = const.tile([128, NKC, 2 * QSL], fp8)
            nc.sync.dma_start(mkadd[:], mT_d.rearrange("(kc p) jq -> p kc jq",
                                                       p=128))
            mkf = const.tile([128, NKC, QSL], bf16)
            nc.sync.dma_start(mkf[:], mf_d.rearrange("(kc p) q -> p kc q",
                                                     p=128))
            magic = const.tile([128, NQS], u32)
            nc.vector.memset(magic[:], 0x5F3759DF)

            # ---- projections (bf16 in, fp8/bf16 out, fp32 psum) ----
            # Only chunk 0's qT/kT are emitted up front (the minimum to
            # start the exp stream); everything else is deferred into the
            # early phases' kc slots via setup_items.
            qT = const.tile([128, NCH, QSL], bf16)
            kT = const.tile([128, NCH, L], bf16)
            vp = const.tile([128, NKC, H, AVP], bf16)
            nc.vector.memset(vp[:, :, :, 64:AVP], 1.0)

            def proj_q(c):
                def emit():
                    ps = psA.tile([128, 2, QSL], f32, tag="sps", name="ps")
                    nc.tensor.matmul(ps[:, 0, :],
                                     w_sb[:, c * 128:(c + 1) * 128],
                                     xqT[:], start=True, stop=True)
                    nc.vector.tensor_copy(qT[:, c, :], ps[:, 0, :])
                return emit

            def proj_k(c, h2):
                def emit():
                    ps = psA.tile([128, 2, QSL], f32, tag="sps", name="ps")
                    for s in range(2):
                        nc.tensor.matmul(
                            ps[:, s, :],
                            w_sb[:, ED + c * 128:ED + (c + 1) * 128],
                            xsT[:, (h2 * 2 + s) * 512:(h2 * 2 + s + 1) * 512],
                            start=True, stop=True)
                    nc.vector.tensor_copy(
                        kT[:, c, h2 * 1024:(h2 + 1) * 1024],
                        ps[:].rearrange("p s q -> p (s q)"))
                return emit

            def proj_v(st2):
                def emit():
                    ps = psA.tile([128, 2, QSL], f32, tag="sps", name="ps")
                    for s in range(2):
                        st = st2 * 2 + s
                        nc.tensor.matmul(ps[:, s, :],
                                         xsT[:, st * 128:(st + 1) * 128],
                                         w_sb[:, 2 * ED:3 * ED],
                                         start=True, stop=True)
                    for s in range(2):
                        st = st2 * 2 + s
                        nc.vector.tensor_copy(
                            vp[:, st, :, 0:64],
                            ps[:, s, :].rearrange("p (h d) -> p h d", h=H))
                return emit

            for f in (proj_q(0), proj_k(0, 0), proj_k(0, 1)):
                f()
            # deferred: vp during phase 0 (used by AV from phase 1 on),
            # chunk c's qT/kT during phase 2c-2 (used from phase 2c on)
            setup_items = {
                0: [proj_v(s) for s in range(NKC // 2)],
                1: [proj_q(1), proj_k(1, 0), proj_k(1, 1)],
                2: [proj_q(2), proj_k(2, 0), proj_k(2, 1)],
                3: [proj_q(3), proj_k(3, 0), proj_k(3, 1)],
            }

            out_ps = outp.tile([128, NQS, 128], f32, tag="op", name="out_ps")
            n_outmm = [0]

            def pass1(i, kc, eb):
                c, p = divmod(i, 2)
                pemask = kc in PEMASK_KCS
                sps = psA.tile([128, 2, QSL], f32, tag="sps", name="sps")
                # QK first (the two row-tiled matmuls run concurrently and
                # open their banks' accumulation groups); for PE-masked kcs
                # the full-array mask-add matmuls accumulate afterwards.
                for j in range(2):
                    r = 64 * p + 32 * j
                    nc.tensor.matmul(
                        sps[:, j, :],
                        kT[r:r + 32, c, kc * 128:(kc + 1) * 128],
                        qT[r:r + 32, c, :],
                        start=True, stop=(not pemask), tile_position=(r, 0),
                        skip_group_check=True)
                if pemask:
                    for j in range(2):
                        nc.tensor.matmul(sps[:, j, :], idm[:],
                                         mkadd[:, kc, j * QSL:(j + 1) * QSL],
                                         start=False, stop=True,
                                         skip_group_check=True)
                if kc in SCHRAUD_KCS:
                    nc.vector.tensor_scalar(eb.bitcast(i16)[:, kc, :, :],
                                            sps[:], SCH_A, SCH_B,
                                            ALU.mult, ALU.add)
                else:
                    nc.scalar.activation(eb[:, kc, :, :], sps[:], AF.Exp)
                if not pemask:
                    mb = mkf[:, kc, :].unsqueeze(1).broadcast_to([128, 2, QSL])
                    nc.vector.tensor_tensor(eb[:, kc, :, :], eb[:, kc, :, :],
                                            mb, ALU.mult)

            def av_step(i, kc, eb, av):
                for j in range(2):
                    nc.tensor.matmul(av[j][:], vp[:, kc, i, :],
                                     eb[:, kc, j, :],
                                     start=(kc == 0), stop=(kc == NKC - 1))

            def epilogue(i, av):
                nat = []
                for j in range(2):
                    a_sb = tmpp.tile([AVP, QSL], bf16, tag=f"a_sb{j}")
                    nc.scalar.copy(a_sb[:], av[j][:])
                    nt = natp.tile([128, NQS, AVP], bf16, tag=f"nat{j}",
                                   name=f"nat{j}")
                    nc.sync.dma_start_transpose(nt[:], a_sb[:])
                    nat.append(nt)

                r0v = tinyp.tile([128, NQS, 1], f32, tag="r0v")
                nc.vector.reciprocal(r0v[:], nat[0][:, :, 64:65])
                r1v = tinyp.tile([128, NQS, 1], f32, tag="r1v")
                nc.vector.reciprocal(r1v[:], nat[1][:, :, 64:65])
                r1p = tinyp.tile([128, NQS, 1], f32, tag="r1p")
                nc.vector.tensor_scalar(r1p[:], r1v[:], nlam_sb[:], None,
                                        ALU.mult)

                t0 = tmpp.tile([128, NQS, 64], f32, tag="t0")
                nc.vector.tensor_tensor(
                    t0[:], nat[0][:, :, 0:64],
                    r0v[:].broadcast_to([128, NQS, 64]), ALU.mult)
                t1 = tmpp.tile([128, NQS, 64], f32, tag="t1")
                nc.vector.tensor_tensor(
                    t1[:], nat[1][:, :, 0:64],
                    r1p[:].broadcast_to([128, NQS, 64]), ALU.mult)
                at4 = tmpp.tile([128, NQS, 64], bf16, tag="at4")
                nc.vector.tensor_tensor(at4[:], t0[:], t1[:], ALU.add)
                sq4 = tmpp.tile([128, NQS, 64], f32, tag="sq4")
                nc.vector.tensor_tensor(sq4[:], at4[:], at4[:], ALU.mult)
                ss4 = tinyp.tile([128, NQS], f32, tag="ss4")
                nc.vector.tensor_reduce(ss4[:], sq4[:],
                                        mybir.AxisListType.X, ALU.add)

                # rr4 = 1/sqrt(ss4/64): fast inverse sqrt on DVE
                msx = tinyp.tile([128, NQS], f32, tag="msx")
                nc.vector.tensor_scalar(msx[:], ss4[:], 1.0 / 64, None,
                                        ALU.mult)
                sh = tinyp.tile([128, NQS], u32, tag="sh")
                nc.vector.tensor_scalar(sh[:], msx[:].bitcast(u32), 1,
                                        None, ALU.logical_shift_right)
                rr4 = tinyp.tile([128, NQS], f32, tag="rr4")
                nc.vector.tensor_tensor(rr4[:].bitcast(u32), magic[:],
                                        sh[:], ALU.subtract)
                nwu = tinyp.tile([128, NQS], f32, tag="nwu")
                nww = tinyp.tile([128, NQS], f32, tag="nww")
                for _ in range(1):
                    nc.vector.tensor_tensor(nwu[:], rr4[:], rr4[:], ALU.mult)
                    nc.vector.scalar_tensor_tensor(
                        nwu[:], nwu[:], 0.5, msx[:], ALU.mult, ALU.mult)
                    nc.vector.tensor_scalar(nww[:], nwu[:], -1.0, 1.5,
                                            ALU.mult, ALU.add)
                    nc.vector.tensor_tensor(rr4[:], rr4[:], nww[:], ALU.mult)

                # at_s cols 64:128 are junk; the transposed junk rows are
                # never read (projection lhsT slice [0:64])
                at_s = tmpp.tile([128, NQS, 128], bf16, tag="at_s")
                nc.vector.tensor_tensor(
                    at_s[:, :, 0:64], at4[:],
                    rr4[:].unsqueeze(2).broadcast_to([128, NQS, 64]),
                    ALU.mult)
                atT = natp.tile([128, NQS, 128], bf16, tag="atT",
                                name=f"atT{i}")
                nc.sync.dma_start_transpose(atT[:], at_s[:])
                for q in range(NQS):
                    # single accumulation group for the whole bank: start
                    # clears has_written bank-wide
                    nc.tensor.matmul(out_ps[:, q, :], atT[0:64, q, :],
                                     wo_sb[:, i, :],
                                     start=(n_outmm[0] == 0),
                                     stop=(n_outmm[0] == H * NQS - 1))
                    n_outmm[0] += 1

            # ---- attention: software-pipelined over 8 pairs ----
            # phase i: pass1(i) + AV(i-1); the last pair's AV runs inline
            # in phase 7 (after its own mask) so there is no drain phase.
            ebufs, avbufs = {}, {}
            for i in range(H + 1):
                if i < H:
                    ebufs[i] = epool.tile([128, NKC, 2, QSL], bf16,
                                          tag="eall", name=f"eall{i}")
                    avbufs[i] = [avp.tile([AVP, QSL], f32, tag=f"av{j}",
                                          name=f"av{j}_{i}")
                                 for j in range(2)]
                items = setup_items.get(i, [])
                for kc in range(NKC):
                    if i < H:
                        pass1(i, kc, ebufs[i])
                    if items and kc % 2 == 0 and kc // 2 < len(items):
                        items[kc // 2]()
                    if i >= 1:
                        av_step(i - 1, kc, ebufs[i - 1], avbufs[i - 1])
                if i >= 1:
                    epilogue(i - 1, avbufs[i - 1])
                    del ebufs[i - 1], avbufs[i - 1]

            out_sb = const.tile([128, NQS, 128], f32)
            nc.vector.tensor_copy(out_sb[:], out_ps[:])
            nc.sync.dma_start(out_d.rearrange("(s p) o -> p s o", p=128),
                              out_sb[:])

    nc.compile()
    return nc


def kernel(**inputs):
    import ml_dtypes
    from concourse.bass_utils import run_bass_kernel_spmd

    bfdt = ml_dtypes.bfloat16
    f8dt = ml_dtypes.float8_e4m3

    x = np.asarray(inputs["x"], np.float32)
    mask = np.asarray(inputs["mask_2d"])
    Wq = np.asarray(inputs["Wq"], np.float32)
    Wkv = np.asarray(inputs["Wkv"], np.float32)
    Wout = np.asarray(inputs["Wout"], np.float32)
    lq1 = np.asarray(inputs["lambda_q1"], np.float32)
    lk1 = np.asarray(inputs["lambda_k1"], np.float32)
    lq2 = np.asarray(inputs["lambda_q2"], np.float32)
    lk2 = np.asarray(inputs["lambda_k2"], np.float32)
    gamma = np.asarray(inputs["gamma"], np.float32)

    lam = float(np.exp(np.sum(lq1 * lk1)) - np.exp(np.sum(lq2 * lk2))
                + LAMBDA_INIT)
    Wq_s = (Wq * DH ** -0.5).astype(np.float32)
    Wk = Wkv[:, :ED]
    Wv = Wkv[:, ED:]

    def pack_heads4(Wm):
        # chunk c (128 cols) holds heads 4c..4c+3 at col offsets 0/32/64/96
        out = np.empty((IN_DIM, NCH * 128), np.float32)
        for n in range(NH):
            c, r = divmod(n, 4)
            out[:, c * 128 + r * 32:c * 128 + r * 32 + 32] = \
                Wm[:, n * DH:(n + 1) * DH]
        return out

    W = np.concatenate([pack_heads4(Wq_s), pack_heads4(Wk), Wv],
                       axis=1).astype(bfdt)
    gs = (gamma * (1.0 - LAMBDA_INIT)).astype(np.float32)
    Wog = (Wout * np.tile(gs, H)[:, None])
    wo = np.ascontiguousarray(
        Wog.reshape(H, 64, OUT_DIM).transpose(1, 0, 2)).astype(bfdt)
    nlam = np.full((128, 1), -lam, np.float32)
    idm = np.eye(128, dtype=np.float32).astype(f8dt)

    xsT = [np.ascontiguousarray(x[b, 0].T).astype(bfdt) for b in range(B)]
    # additive mask [keys, queries] (PE path) and multiplicative (DVE path)
    maskT, maskF = [], []
    for b in range(B):
        madd = np.where(mask[b].T, 0.0, MASK_NEG).astype(np.float32)  # [L, L]q
        maskT.append(madd)
        maskF.append(mask[b].T.astype(np.float32))

    if "nc" not in _CACHE:
        _CACHE["nc"] = _build_program()
    nc = _CACHE["nc"]

    in_maps = []
    for core in range(8):
        b, qc = divmod(core, 4)
        msl = maskT[b][:, qc * QSL:(qc + 1) * QSL]
        mdup = np.ascontiguousarray(
            np.concatenate([msl, msl], axis=1)).astype(f8dt)
        in_maps.append({
            "xsT": xsT[b],
            "xqT": np.ascontiguousarray(
                xsT[b][:, qc * QSL:(qc + 1) * QSL]),
            "maskT": mdup,
            "maskF": np.ascontiguousarray(
                maskF[b][:, qc * QSL:(qc + 1) * QSL]).astype(bfdt),
            "idm": idm,
            "w": W,
            "wo": wo,
            "nlam": nlam,
        })

    r = run_bass_kernel_spmd(nc, in_maps, list(range(8)))
    _CACHE["last_results"] = r
    res = r.results

    out = np.empty((B, 1, L, OUT_DIM), np.float32)
    for core in range(8):
        b, qc = divmod(core, 4)
        out[b, 0, qc * QSL:(qc + 1) * QSL, :] = res[core]["out"]
    return out
